# revision 31
# baseline (speedup 1.0000x reference)
"""Trainium2 Bass kernel for nn_GRUCell_21612275433682.

Math (from the reference):
  - h0 = 0, so the W_hh matmul is dead: only b_hh enters the gates.
  - y = x @ W_ih.T            (the single big GEMM, [B*T, I] @ [I, 3H])
  - r = (y_r + b_ih_r + b_hh_r > 0)
  - z = (y_z + b_ih_z + b_hh_z > 0)
  - n = (y_n + b_ih_n + r*b_hh_n > 0)
  - cur = (1-z)*n   in {0,1}
  - LIF over T=4 steps:  mem' = 0.99*mem + cur_t - spk_{t-1};  spk_t = (mem' > 1)
    spk_0 is identically 0 (mem1 = cur0 <= 1).

Strategy: pure data parallel over 8 cores (B sharded 256/core). Per core one
[3H=6144, TB=1024] x [I=2048] GEMM with W stationary ([I,3H] tiles) and X
moving.  Moving-column layout is n-chunk-major / b-major-within-chunk:
col = n*512 + t*128 + blo  (b = n*128 + blo), so each 512-wide n-tile holds
all 4 timesteps of 128 batch rows -- the LIF scan and the output DMA are
self-contained per n-tile (short serial tail after the last matmul).

GEMM precision scheme ("f16f8"):
  W,X split into fp16 hi/lo; 1 fp16 pass (hi*hi, products exact in fp32
  PSUM) + both cross terms (hi*lo + lo*hi) packed into one fp8e4m3
  DoubleRow pass.  Everything is pre-scaled by powers of two to a common
  2^16 PSUM scale so all passes accumulate into one bank; the gate
  thresholds absorb the scale.  The r-gate skips the fp8 correction (an
  r flip only matters when y_n lands inside the +-b_hn window, ~1.5%).

Schedule notes (from perfetto/NTFF analysis of the previous version):
  - Every PE matmul instruction at FD=512 costs ~233-237ns regardless of
    dtype/perf-mode, so runtime ~= 5 MM/(j,n,k-tile) * 233ns.  The
    instruction count is minimal for the precision budget; what's left is
    head/tail/HAM-ramp trimming:
  - X DMA is issued in (n-tile, k-chunk) consumption order; the old
    k-major order starved the PE mid-j0 (HAM dropped to K=4/8 for ~14us).
  - W for j=0 is k-chunked so the first real matmul starts ~2us in.
  - LIF + out DMA per (j,n) shrinks the post-last-matmul serial tail.
"""

import numpy as np
import ml_dtypes

BF16 = ml_dtypes.bfloat16
FP8 = ml_dtypes.float8_e4m3

# Full problem sizes (hardcoded per contract)
B, I, H, T = 2048, 2048, 2048, 4
NCORES = 8
P = 128
BQ = 128          # batch rows per 512-wide n-tile (4 timesteps each)

SCHEME = "f16f8"

# scheme f16f8 scale choices (powers of two; see product-scale table below)
#   main:  (wh * 2^8) @ (xh * 2^8)            -> y_main * 2^16
#   cross: fp8(wh*2^5) @ fp8(xl*2^11)         -> cross1 * 2^16
#          fp8(wl*2^16) @ fp8(xh)             -> cross2 * 2^16
SW_H, SX_H = 256.0, 256.0
SW8_H, SX8_L = 32.0, 2048.0
SW8_L, SX8_H = 65536.0, 1.0
SCALE = 65536.0

_CACHE = {}

# test-harness knobs (grading path leaves these alone)
TRACE = False
LAST_EXEC_NS = None
LAST_RESULTS = None


def build_nc(KT, GJ, BT):
    """Build the per-core Bass program.

    KT: number of 128-wide K tiles (I = 128*KT)
    GJ: number of 128-row h-tile groups per gate (H = 128*GJ)
    BT: batch rows per timestep per core (TB = 4*BT total moving columns)
    """
    import concourse.mybir as mybir
    import concourse.tile as tile
    from concourse import bacc

    TB = 4 * BT
    NT = TB // 512
    assert NT * 512 == TB and BT % BQ == 0

    f32 = mybir.dt.float32
    f16 = mybir.dt.float16
    f8 = mybir.dt.float8e4
    A = mybir.AluOpType
    DR = mybir.MatmulPerfMode.DoubleRow

    nc = bacc.Bacc("TRN2", target_bir_lowering=False, debug=False,
                   num_devices=NCORES)

    xh_d = nc.dram_tensor("xh", [NT, P, KT, 512], f16, kind="ExternalInput")
    x8_d = nc.dram_tensor("x8", [NT, P, KT, 512], f8, kind="ExternalInput")
    wh_d = nc.dram_tensor("wh", [GJ, P, KT, 3, P], f16, kind="ExternalInput")
    w8_d = nc.dram_tensor("w8", [GJ, P, KT, 2, 2, P], f8,
                          kind="ExternalInput")
    br_d = nc.dram_tensor("br", [P, GJ], f32, kind="ExternalInput")
    bz_d = nc.dram_tensor("bz", [P, GJ], f32, kind="ExternalInput")
    bin_d = nc.dram_tensor("bin", [P, GJ], f32, kind="ExternalInput")
    bhn_d = nc.dram_tensor("bhn", [P, GJ], f32, kind="ExternalInput")
    out_d = nc.dram_tensor("out", [GJ, NT, P, 3 * BQ], f32,
                           kind="ExternalOutput")

    with tile.TileContext(nc) as tc:
        with (
            tc.tile_pool(name="xp", bufs=1) as xp,
            tc.tile_pool(name="wp", bufs=2) as wp,
            tc.tile_pool(name="bp", bufs=1) as bp,
            tc.tile_pool(name="gp", bufs=2) as gp,
            tc.tile_pool(name="lp", bufs=2) as lp,
            tc.tile_pool(name="op", bufs=2) as op,
            tc.tile_pool(name="pp", bufs=7, space="PSUM") as pp,
        ):
            # The start window (X for both n-tiles + W for j=0,1) is
            # HBM-bandwidth-bound: j0-n0 consumes ~360KB per 1.17us
            # k-tile, about the per-core HBM rate.  X rides the ACT
            # HWDGE ring and W j0/j1 the sync ring, both in graded
            # consumption-order chunks (>=2 k-tiles each -- a dma_start
            # trigger costs ~650ns of queue issue, so per-k chunks cap
            # the ring at ~160GB/s).  The fp8 hi piece of X is derived
            # on-chip from xh (DVE cast) instead of DMA'd: -2.1MB of
            # pull exactly where bandwidth is scarcest.  W for j>=2 is
            # naturally gated to compute pace by the bufs=2 weight pool.
            xh_sb = [xp.tile([P, KT, 512], f16, tag=f"xh{n}",
                             name=f"xh_sb{n}") for n in range(NT)]
            x2_sb = [xp.tile([P, 2, KT, 512], f8, tag=f"x2{n}",
                             name=f"x2_sb{n}") for n in range(NT)]
            w_pre = []
            for j in range(min(2, GJ)):
                whp = wp.tile([P, KT, 3, P], f16, tag="wh",
                              name=f"wh_pre{j}")
                w2p = wp.tile([P, KT, 2, 2, P], f8, tag="w2",
                              name=f"w2_pre{j}")
                w_pre.append((whp, w2p))

            # Trigger issue costs ~650ns of queue time per dma_start, so
            # the start-window streams ride four parallel queues: xh on
            # ACT, x8lo (+derive-casts) on Vector, W j0 then j>=2 on
            # Sync, and W j1 on ACT *behind* X n1 (queue FIFO delays its
            # transfer into the n1 window where HBM bandwidth is slack).
            def x_chunk(n, a, b):
                cs = slice(a, b)
                nc.scalar.dma_start(out=xh_sb[n][:, cs],
                                    in_=xh_d[n][:, cs])
                eng = nc.gpsimd if n == 0 else nc.scalar
                eng.dma_start(out=x2_sb[n][:, 0, cs],
                              in_=x8_d[n][:, cs])
                for k in range(a, b):
                    nc.vector.tensor_scalar(x2_sb[n][:, 1, k],
                                            xh_sb[n][:, k],
                                            1.0 / SW_H, None, A.mult)

            kb0 = [0, 1, 2, 4, 7, 11, KT] if KT == 16 else [0, KT]
            kbn = [0, 4, 8, KT] if KT == 16 else [0, KT]
            for i, (a, b) in enumerate(zip(kb0[:-1], kb0[1:])):
                x_chunk(0, a, b)
                nc.sync.dma_start(out=w_pre[0][0][:, a:b],
                                  in_=wh_d[0][:, a:b])
                nc.gpsimd.dma_start(out=w_pre[0][1][:, a:b],
                                     in_=w8_d[0][:, a:b])
            for n in range(1, NT):
                for a, b in zip(kbn[:-1], kbn[1:]):
                    x_chunk(n, a, b)
            if GJ > 1:
                nc.scalar.dma_start(out=w_pre[1][0][:], in_=wh_d[1])
                nc.scalar.dma_start(out=w_pre[1][1][:], in_=w8_d[1])

            # Warm the PE (HAM un-throttle needs ~3.4us of sustained
            # matmul activity) while the first input DMAs land.
            warm = bp.tile([P, 512], f16, tag="warm")
            nc.vector.memset(warm[:], 0)
            wps = pp.tile([P, 512], f32, tag="warmps", name="warmps",
                          bufs=1)
            def warm_fill(cnt):
                # PE-queue filler: keeps HAM ramped and absorbs known
                # DMA-feed deficits without delaying later real MMs.
                for r_ in range(cnt):
                    nc.tensor.matmul(wps[:, 0:256], warm[:, 0:P],
                                     warm[:, 0:256], start=(r_ == 0),
                                     stop=(r_ == cnt - 1),
                                     skip_group_check=True)

            warm_fill(28)

            br_sb = bp.tile([P, GJ], f32, tag="br")
            nc.gpsimd.dma_start(out=br_sb[:], in_=br_d[:])
            bz_sb = bp.tile([P, GJ], f32, tag="bz")
            nc.gpsimd.dma_start(out=bz_sb[:], in_=bz_d[:])
            bin_sb = bp.tile([P, GJ], f32, tag="bin")
            nc.gpsimd.dma_start(out=bin_sb[:], in_=bin_d[:])
            bhn_sb = bp.tile([P, GJ], f32, tag="bhn")
            nc.gpsimd.dma_start(out=bhn_sb[:], in_=bhn_d[:])

            for j in range(GJ):
                if j < len(w_pre):
                    wh_sb, w2_sb = w_pre[j]
                else:
                    wh_sb = wp.tile([P, KT, 3, P], f16, tag="wh")
                    w2_sb = wp.tile([P, KT, 2, 2, P], f8, tag="w2")
                    nc.sync.dma_start(out=wh_sb[:], in_=wh_d[j])
                    nc.sync.dma_start(out=w2_sb[:], in_=w8_d[j])

                for n in range(NT):
                    # Alternate fp16 MMs with fp8-DR MMs across the 3
                    # PSUM banks of this n-tile so every 256-col DR
                    # weight-load hides under a preceding fp16 MM.
                    # g=0 (r-gate) skips the fp8 correction.
                    # The very last (j,n) cell instead runs gate-major
                    # (all of g=0, then g=1, then g=2) so the r/z DVE
                    # work overlaps the remaining matmuls and the
                    # serial tail after the final MM is just the n-gate
                    # compare + LIF.
                    ps = [pp.tile([P, 512], f32, tag="ps",
                                  name=f"ps_{j}_{n}_{g}")
                          for g in range(3)]
                    last_cell = (j == GJ - 1 and n == NT - 1)
                    if last_cell:
                        order = [(k, g) for g in range(3) for k in range(KT)]
                    else:
                        order = [(k, g) for k in range(KT) for g in range(3)]
                    if j == 0 and n == 1:
                        warm_fill(4)
                    for k, g in order:
                        nc.tensor.matmul(ps[g][:], wh_sb[:, k, g, :],
                                         xh_sb[n][:, k],
                                         start=(k == 0),
                                         stop=(g == 0 and k == KT - 1),
                                         skip_group_check=True)
                        if g != 0:
                            nc.tensor.matmul(ps[g][:],
                                             w2_sb[:, k, g - 1],
                                             x2_sb[n][:, :, k, :],
                                             perf_mode=DR,
                                             start=False,
                                             stop=(k == KT - 1),
                                             skip_group_check=True)

                    # Gates: psum holds y*2^16; br/bz arrive pre-scaled
                    # by -2^16 so the compare absorbs bias and scale.
                    bj = lambda t: t[:, j:j + 1]
                    r = gp.tile([P, 512], f32, tag="r")
                    zb = gp.tile([P, 512], f32, tag="zb")
                    nc.vector.tensor_scalar(r[:], ps[0][:], bj(br_sb),
                                            None, A.is_gt)
                    nc.vector.tensor_scalar(zb[:], ps[1][:], bj(bz_sb),
                                            None, A.is_le)
                    # rbn = r*b_hn + b_in ; n2 = y_n*2^-16 + rbn
                    rbn = gp.tile([P, 512], f32, tag="rbn")
                    nc.vector.tensor_scalar(rbn[:], r[:], bj(bhn_sb),
                                            bj(bin_sb), A.mult, A.add)
                    n2 = gp.tile([P, 512], f32, tag="n2")
                    nc.vector.scalar_tensor_tensor(n2[:], ps[2][:],
                                                   1.0 / SCALE, rbn[:],
                                                   A.mult, A.add)
                    cur = gp.tile([P, 512], f32, tag="cur")
                    nc.vector.scalar_tensor_tensor(cur[:], n2[:], 0.0,
                                                   zb[:], A.is_gt, A.mult)

                    # LIF over the 4 timesteps (t-major within the
                    # n-tile: col = t*BQ + blo), self-contained here.
                    # With cur in {0,1}, beta=0.99, thr=1, T=4 the scan
                    # has a closed boolean form:
                    #   s1 = c0&c1; s2 = c2&(c0|c1); s3 = c3&(c0|c1|c2)
                    out_sb = op.tile([P, 3 * BQ], f32, tag="out")
                    c0 = cur[:, 0 * BQ:1 * BQ]
                    c1 = cur[:, 1 * BQ:2 * BQ]
                    c2 = cur[:, 2 * BQ:3 * BQ]
                    c3 = cur[:, 3 * BQ:4 * BQ]
                    s1 = out_sb[:, 0 * BQ:1 * BQ]
                    s2 = out_sb[:, 1 * BQ:2 * BQ]
                    s3 = out_sb[:, 2 * BQ:3 * BQ]

                    a01 = lp.tile([P, BQ], f32, tag="a01")
                    nc.vector.tensor_tensor(a01[:], c0, c1, A.add)
                    nc.vector.tensor_scalar(s1, a01[:], 1.0, None, A.is_gt)
                    nc.vector.scalar_tensor_tensor(s2, a01[:], 0.0, c2,
                                                   A.is_gt, A.mult)
                    a02 = lp.tile([P, BQ], f32, tag="a02")
                    nc.vector.tensor_tensor(a02[:], a01[:], c2, A.add)
                    nc.vector.scalar_tensor_tensor(s3, a02[:], 0.0, c3,
                                                   A.is_gt, A.mult)

                    nc.sync.dma_start(out=out_d[j, n], in_=out_sb[:])

    nc.compile()
    return nc


def _blocked_w(Wt, KT, GJ):
    """[I, 3H] -> (j, p, k, g, m) blocked layout (k-major for chunked DMA)."""
    Wb = Wt.reshape(KT, P, 3, GJ, P).transpose(3, 1, 0, 2, 4)
    return np.ascontiguousarray(Wb)


def prep_weights(W_ih, b_ih, b_hh, KT, GJ):
    """Host-side packing of weights/biases (shared across cores)."""
    threeH = 3 * GJ * P
    II = KT * P
    Wt = np.ascontiguousarray(W_ih[:threeH, :II].T)          # [I, 3H] fp32

    HH = GJ * P
    b_r = (b_ih[0:HH] + b_hh[0:HH]).astype(np.float32)
    b_z = (b_ih[HH:2 * HH] + b_hh[HH:2 * HH]).astype(np.float32)
    b_in = b_ih[2 * HH:3 * HH].astype(np.float32)
    b_hn = b_hh[2 * HH:3 * HH].astype(np.float32)
    asb = lambda b: np.ascontiguousarray(b.reshape(GJ, P).T)

    Wb = _blocked_w(Wt, KT, GJ)                              # [GJ,P,KT,3,P]
    wh16 = Wb.astype(np.float16)
    wl = Wb - wh16.astype(np.float32)
    wh_scaled = (wh16.astype(np.float32) * SW_H).astype(np.float16)
    # fp8 correction factors only for the z/n gates (g=1,2)
    w8 = np.empty((Wb.shape[0], P, KT, 2, 2, P), dtype=FP8)
    w8[:, :, :, :, 0, :] = (wh16[:, :, :, 1:3].astype(np.float32)
                            * SW8_H).astype(FP8)
    w8[:, :, :, :, 1, :] = (wl[:, :, :, 1:3] * SW8_L).astype(FP8)
    return {"wh": wh_scaled, "w8": w8,
            "br": asb(-b_r * SCALE), "bz": asb(-b_z * SCALE),
            "bin": asb(b_in), "bhn": asb(b_hn)}


def prep_x(x_core, KT, BT):
    """x_core: [BL, I, T] fp32 -> per-core input dict.

    Moving-column layout: col = n*512 + t*BQ + blo with b = n*BQ + blo.
    """
    II = KT * P
    NT = BT // BQ
    a = x_core[:, :II, :].reshape(NT, BQ, II, 4)       # [n, blo, i, t]
    a = a.transpose(2, 0, 3, 1).reshape(II, NT, 512)   # [i, n, t*BQ+blo]
    xt = np.ascontiguousarray(
        a.reshape(KT, P, NT, 512).transpose(2, 1, 0, 3))  # [NT, P, KT, 512]
    xh16 = xt.astype(np.float16)
    xl = xt - xh16.astype(np.float32)
    xh_scaled = (xh16.astype(np.float32) * SX_H).astype(np.float16)
    x8 = (xl * SX8_L).astype(FP8)          # lo piece; hi derived on-chip
    return {"xh": xh_scaled, "x8": x8}


def unpack_out(out, GJ, BT):
    """out: [GJ, NT, P, 3*BQ] fp32 -> spikes [BL, H', 4] with t=0 zeros."""
    HH = GJ * P
    NT = BT // BQ
    arr = out.reshape(GJ, NT, P, 3, BQ)              # [j, n, p, t-1, blo]
    res = np.zeros((BT, HH, 4), dtype=np.float32)
    # res[n*BQ+blo, j*P+p, 1+ti] = arr[j, n, p, ti, blo]
    res[:, :, 1:4] = arr.transpose(1, 4, 0, 2, 3).reshape(BT, HH, 3)
    return res


def kernel(inputs, W_ih, b_ih, W_hh, b_hh):
    from concourse.bass_utils import run_bass_kernel_spmd

    # BT = batch rows per timestep per core (= local batch size BL)
    KT, GJ, BT = I // P, H // P, B // NCORES
    key = (KT, GJ, BT, SCHEME)
    if key not in _CACHE:
        _CACHE[key] = build_nc(KT, GJ, BT)
    nc = _CACHE[key]

    wmap = prep_weights(np.asarray(W_ih, dtype=np.float32),
                        np.asarray(b_ih, dtype=np.float32),
                        np.asarray(b_hh, dtype=np.float32), KT, GJ)

    x = np.asarray(inputs, dtype=np.float32)
    in_maps = []
    BL = B // NCORES
    for c in range(NCORES):
        m = dict(wmap)
        m.update(prep_x(x[c * BL:(c + 1) * BL], KT, BT))
        in_maps.append(m)

    res = run_bass_kernel_spmd(nc, in_maps, list(range(NCORES)), trace=TRACE)
    global LAST_EXEC_NS, LAST_RESULTS
    LAST_EXEC_NS = res.exec_time_ns
    LAST_RESULTS = res

    out = np.empty((B, H, T), dtype=np.float32)
    for c in range(NCORES):
        out[c * BL:(c + 1) * BL] = unpack_out(res.results[c]["out"], GJ, BT)
    return out


# revision 32
# speedup vs baseline: 1.0054x; 1.0054x over previous
"""Trainium2 Bass kernel for nn_GRUCell_21612275433682.

Math (from the reference):
  - h0 = 0, so the W_hh matmul is dead: only b_hh enters the gates.
  - y = x @ W_ih.T            (the single big GEMM, [B*T, I] @ [I, 3H])
  - r = (y_r + b_ih_r + b_hh_r > 0)
  - z = (y_z + b_ih_z + b_hh_z > 0)
  - n = (y_n + b_ih_n + r*b_hh_n > 0)
  - cur = (1-z)*n   in {0,1}
  - LIF over T=4 steps:  mem' = 0.99*mem + cur_t - spk_{t-1};  spk_t = (mem' > 1)
    spk_0 is identically 0 (mem1 = cur0 <= 1).

Strategy: pure data parallel over 8 cores (B sharded 256/core). Per core one
[3H=6144, TB=1024] x [I=2048] GEMM with W stationary ([I,3H] tiles) and X
moving.  Moving-column layout is n-chunk-major / b-major-within-chunk:
col = n*512 + t*128 + blo  (b = n*128 + blo), so each 512-wide n-tile holds
all 4 timesteps of 128 batch rows -- the LIF scan and the output DMA are
self-contained per n-tile (short serial tail after the last matmul).

GEMM precision scheme ("f16f8"):
  W,X split into fp16 hi/lo; 1 fp16 pass (hi*hi, products exact in fp32
  PSUM) + both cross terms (hi*lo + lo*hi) packed into one fp8e4m3
  DoubleRow pass.  Everything is pre-scaled by powers of two to a common
  2^16 PSUM scale so all passes accumulate into one bank; the gate
  thresholds absorb the scale.  The r-gate skips the fp8 correction (an
  r flip only matters when y_n lands inside the +-b_hn window, ~1.5%).

Schedule notes (from perfetto/NTFF analysis of the previous version):
  - Every PE matmul instruction at FD=512 costs ~233-237ns regardless of
    dtype/perf-mode, so runtime ~= 5 MM/(j,n,k-tile) * 233ns.  The
    instruction count is minimal for the precision budget; what's left is
    head/tail/HAM-ramp trimming:
  - X DMA is issued in (n-tile, k-chunk) consumption order; the old
    k-major order starved the PE mid-j0 (HAM dropped to K=4/8 for ~14us).
  - W for j=0 is k-chunked; ~28 warm-up matmuls cover the DMA-transient
    so the real MM stream starts fed and never re-stalls (a PE idle gap
    makes HAM halve the clock for 3.4us+).
  - LIF + out DMA per (j,n) shrinks the post-last-matmul serial tail.
"""

import numpy as np
import ml_dtypes

BF16 = ml_dtypes.bfloat16
FP8 = ml_dtypes.float8_e4m3

# Full problem sizes (hardcoded per contract)
B, I, H, T = 2048, 2048, 2048, 4
NCORES = 8
P = 128
BQ = 128          # batch rows per 512-wide n-tile (4 timesteps each)

SCHEME = "f16f8"

# scheme f16f8 scale choices (powers of two; see product-scale table below)
#   main:  (wh * 2^8) @ (xh * 2^8)            -> y_main * 2^16
#   cross: fp8(wh*2^5) @ fp8(xl*2^11)         -> cross1 * 2^16
#          fp8(wl*2^16) @ fp8(xh)             -> cross2 * 2^16
SW_H, SX_H = 256.0, 256.0
SW8_H, SX8_L = 32.0, 2048.0
SW8_L, SX8_H = 65536.0, 1.0
SCALE = 65536.0

_CACHE = {}

# test-harness knobs (grading path leaves these alone)
TRACE = False
LAST_EXEC_NS = None
LAST_RESULTS = None


def build_nc(KT, GJ, BT):
    """Build the per-core Bass program.

    KT: number of 128-wide K tiles (I = 128*KT)
    GJ: number of 128-row h-tile groups per gate (H = 128*GJ)
    BT: batch rows per timestep per core (TB = 4*BT total moving columns)
    """
    import concourse.mybir as mybir
    import concourse.tile as tile
    from concourse import bacc

    TB = 4 * BT
    NT = TB // 512
    assert NT * 512 == TB and BT % BQ == 0

    f32 = mybir.dt.float32
    f16 = mybir.dt.float16
    f8 = mybir.dt.float8e4
    A = mybir.AluOpType
    DR = mybir.MatmulPerfMode.DoubleRow

    nc = bacc.Bacc("TRN2", target_bir_lowering=False, debug=False,
                   num_devices=NCORES)

    xh_d = nc.dram_tensor("xh", [NT, P, KT, 512], f16, kind="ExternalInput")
    x8_d = nc.dram_tensor("x8", [NT, P, KT, 512], f8, kind="ExternalInput")
    wh_d = nc.dram_tensor("wh", [GJ, P, KT, 3, P], f16, kind="ExternalInput")
    w8_d = nc.dram_tensor("w8", [GJ, P, KT, 2, 2, P], f8,
                          kind="ExternalInput")
    br_d = nc.dram_tensor("br", [P, GJ], f32, kind="ExternalInput")
    bz_d = nc.dram_tensor("bz", [P, GJ], f32, kind="ExternalInput")
    bin_d = nc.dram_tensor("bin", [P, GJ], f32, kind="ExternalInput")
    bhn_d = nc.dram_tensor("bhn", [P, GJ], f32, kind="ExternalInput")
    out_d = nc.dram_tensor("out", [GJ, NT, P, 3 * BQ], f32,
                           kind="ExternalOutput")

    with tile.TileContext(nc) as tc:
        with (
            tc.tile_pool(name="xp", bufs=1) as xp,
            tc.tile_pool(name="wp", bufs=2) as wp,
            tc.tile_pool(name="bp", bufs=1) as bp,
            tc.tile_pool(name="gp", bufs=2) as gp,
            tc.tile_pool(name="lp", bufs=2) as lp,
            tc.tile_pool(name="op", bufs=2) as op,
            tc.tile_pool(name="pp", bufs=7, space="PSUM") as pp,
        ):
            # The start window (X for both n-tiles + W for j=0,1) is
            # HBM-bandwidth-bound: j0-n0 consumes ~360KB per 1.17us
            # k-tile, about the per-core HBM rate.  X rides the ACT
            # HWDGE ring and W j0/j1 the sync ring, both in graded
            # consumption-order chunks (>=2 k-tiles each -- a dma_start
            # trigger costs ~650ns of queue issue, so per-k chunks cap
            # the ring at ~160GB/s).  The fp8 hi piece of X is derived
            # on-chip from xh (DVE cast) instead of DMA'd: -2.1MB of
            # pull exactly where bandwidth is scarcest.  W for j>=2 is
            # naturally gated to compute pace by the bufs=2 weight pool.
            xh_sb = [xp.tile([P, KT, 512], f16, tag=f"xh{n}",
                             name=f"xh_sb{n}") for n in range(NT)]
            x2_sb = [xp.tile([P, 2, KT, 512], f8, tag=f"x2{n}",
                             name=f"x2_sb{n}") for n in range(NT)]
            w_pre = []
            for j in range(min(2, GJ)):
                whp = wp.tile([P, KT, 3, P], f16, tag="wh",
                              name=f"wh_pre{j}")
                w2p = wp.tile([P, KT, 2, 2, P], f8, tag="w2",
                              name=f"w2_pre{j}")
                w_pre.append((whp, w2p))

            # Start-window streams: xh + x8lo on the ACT queue, W j0
            # (k-chunked) then j>=2 on Sync, W j1 on ACT *behind* X n1
            # (queue FIFO delays its transfer into the n1 window where
            # HBM bandwidth is slack).  A dma_start trigger costs ~650ns
            # of queue issue, so chunks stay >= 1-2 k-tiles.
            def x_chunk(n, a, b):
                cs = slice(a, b)
                nc.scalar.dma_start(out=xh_sb[n][:, cs],
                                    in_=xh_d[n][:, cs])
                nc.scalar.dma_start(out=x2_sb[n][:, 0, cs],
                                    in_=x8_d[n][:, cs])
                for k in range(a, b):
                    nc.vector.tensor_scalar(x2_sb[n][:, 1, k],
                                            xh_sb[n][:, k],
                                            1.0 / SW_H, None, A.mult)

            kb0 = [0, 1, 2, 4, 7, 11, KT] if KT == 16 else [0, KT]
            kbn = [0, 4, 8, KT] if KT == 16 else [0, KT]
            for i, (a, b) in enumerate(zip(kb0[:-1], kb0[1:])):
                x_chunk(0, a, b)
                nc.sync.dma_start(out=w_pre[0][0][:, a:b],
                                  in_=wh_d[0][:, a:b])
                nc.sync.dma_start(out=w_pre[0][1][:, a:b],
                                  in_=w8_d[0][:, a:b])
            for n in range(1, NT):
                for a, b in zip(kbn[:-1], kbn[1:]):
                    x_chunk(n, a, b)
            if GJ > 1:
                nc.scalar.dma_start(out=w_pre[1][0][:], in_=wh_d[1])
                nc.scalar.dma_start(out=w_pre[1][1][:], in_=w8_d[1])

            # Warm the PE (HAM un-throttle needs ~3.4us of sustained
            # matmul activity) while the first input DMAs land.
            warm = bp.tile([P, 512], f16, tag="warm")
            nc.vector.memset(warm[:], 0)
            wps = pp.tile([P, 512], f32, tag="warmps", name="warmps",
                          bufs=1)
            def warm_fill(cnt):
                # PE-queue filler: keeps HAM ramped and absorbs known
                # DMA-feed deficits without delaying later real MMs.
                for r_ in range(cnt):
                    nc.tensor.matmul(wps[:, 0:256], warm[:, 0:P],
                                     warm[:, 0:256], start=(r_ == 0),
                                     stop=(r_ == cnt - 1),
                                     skip_group_check=True)

            warm_fill(28)

            br_sb = bp.tile([P, GJ], f32, tag="br")
            nc.gpsimd.dma_start(out=br_sb[:], in_=br_d[:])
            bz_sb = bp.tile([P, GJ], f32, tag="bz")
            nc.gpsimd.dma_start(out=bz_sb[:], in_=bz_d[:])
            bin_sb = bp.tile([P, GJ], f32, tag="bin")
            nc.gpsimd.dma_start(out=bin_sb[:], in_=bin_d[:])
            bhn_sb = bp.tile([P, GJ], f32, tag="bhn")
            nc.gpsimd.dma_start(out=bhn_sb[:], in_=bhn_d[:])

            for j in range(GJ):
                if j < len(w_pre):
                    wh_sb, w2_sb = w_pre[j]
                else:
                    wh_sb = wp.tile([P, KT, 3, P], f16, tag="wh")
                    w2_sb = wp.tile([P, KT, 2, 2, P], f8, tag="w2")
                    nc.sync.dma_start(out=wh_sb[:], in_=wh_d[j])
                    nc.sync.dma_start(out=w2_sb[:], in_=w8_d[j])

                for n in range(NT):
                    # Alternate fp16 MMs with fp8-DR MMs across the 3
                    # PSUM banks of this n-tile so every 256-col DR
                    # weight-load hides under a preceding fp16 MM.
                    # g=0 (r-gate) skips the fp8 correction.
                    # The very last (j,n) cell instead runs gate-major
                    # (all of g=0, then g=1, then g=2) so the r/z DVE
                    # work overlaps the remaining matmuls and the
                    # serial tail after the final MM is just the n-gate
                    # compare + LIF.
                    ps = [pp.tile([P, 512], f32, tag="ps",
                                  name=f"ps_{j}_{n}_{g}")
                          for g in range(3)]
                    last_cell = (j == GJ - 1 and n == NT - 1)
                    if last_cell:
                        order = [(k, g) for g in range(3) for k in range(KT)]
                    else:
                        order = [(k, g) for k in range(KT) for g in range(3)]
                    if j == 0 and n == 1:
                        warm_fill(4)
                    for k, g in order:
                        nc.tensor.matmul(ps[g][:], wh_sb[:, k, g, :],
                                         xh_sb[n][:, k],
                                         start=(k == 0),
                                         stop=(g == 0 and k == KT - 1),
                                         skip_group_check=True)
                        if g != 0:
                            nc.tensor.matmul(ps[g][:],
                                             w2_sb[:, k, g - 1],
                                             x2_sb[n][:, :, k, :],
                                             perf_mode=DR,
                                             start=False,
                                             stop=(k == KT - 1),
                                             skip_group_check=True)

                    # Gates: psum holds y*2^16; br/bz arrive pre-scaled
                    # by -2^16 so the compare absorbs bias and scale.
                    bj = lambda t: t[:, j:j + 1]
                    r = gp.tile([P, 512], f32, tag="r")
                    zb = gp.tile([P, 512], f32, tag="zb")
                    nc.vector.tensor_scalar(r[:], ps[0][:], bj(br_sb),
                                            None, A.is_gt)
                    nc.vector.tensor_scalar(zb[:], ps[1][:], bj(bz_sb),
                                            None, A.is_le)
                    # rbn = r*b_hn + b_in ; n2 = y_n*2^-16 + rbn
                    rbn = gp.tile([P, 512], f32, tag="rbn")
                    nc.vector.tensor_scalar(rbn[:], r[:], bj(bhn_sb),
                                            bj(bin_sb), A.mult, A.add)
                    n2 = gp.tile([P, 512], f32, tag="n2")
                    nc.vector.scalar_tensor_tensor(n2[:], ps[2][:],
                                                   1.0 / SCALE, rbn[:],
                                                   A.mult, A.add)
                    cur = gp.tile([P, 512], f32, tag="cur")
                    nc.vector.scalar_tensor_tensor(cur[:], n2[:], 0.0,
                                                   zb[:], A.is_gt, A.mult)

                    # LIF over the 4 timesteps (t-major within the
                    # n-tile: col = t*BQ + blo), self-contained here.
                    # With cur in {0,1}, beta=0.99, thr=1, T=4 the scan
                    # has a closed boolean form:
                    #   s1 = c0&c1; s2 = c2&(c0|c1); s3 = c3&(c0|c1|c2)
                    out_sb = op.tile([P, 3 * BQ], f32, tag="out")
                    c0 = cur[:, 0 * BQ:1 * BQ]
                    c1 = cur[:, 1 * BQ:2 * BQ]
                    c2 = cur[:, 2 * BQ:3 * BQ]
                    c3 = cur[:, 3 * BQ:4 * BQ]
                    s1 = out_sb[:, 0 * BQ:1 * BQ]
                    s2 = out_sb[:, 1 * BQ:2 * BQ]
                    s3 = out_sb[:, 2 * BQ:3 * BQ]

                    a01 = lp.tile([P, BQ], f32, tag="a01")
                    nc.vector.tensor_tensor(a01[:], c0, c1, A.add)
                    nc.vector.tensor_scalar(s1, a01[:], 1.0, None, A.is_gt)
                    nc.vector.scalar_tensor_tensor(s2, a01[:], 0.0, c2,
                                                   A.is_gt, A.mult)
                    a02 = lp.tile([P, BQ], f32, tag="a02")
                    nc.vector.tensor_tensor(a02[:], a01[:], c2, A.add)
                    nc.vector.scalar_tensor_tensor(s3, a02[:], 0.0, c3,
                                                   A.is_gt, A.mult)

                    nc.sync.dma_start(out=out_d[j, n], in_=out_sb[:])

    nc.compile()
    return nc


def _blocked_w(Wt, KT, GJ):
    """[I, 3H] -> (j, p, k, g, m) blocked layout (k-major for chunked DMA)."""
    Wb = Wt.reshape(KT, P, 3, GJ, P).transpose(3, 1, 0, 2, 4)
    return np.ascontiguousarray(Wb)


def prep_weights(W_ih, b_ih, b_hh, KT, GJ):
    """Host-side packing of weights/biases (shared across cores)."""
    threeH = 3 * GJ * P
    II = KT * P
    Wt = np.ascontiguousarray(W_ih[:threeH, :II].T)          # [I, 3H] fp32

    HH = GJ * P
    b_r = (b_ih[0:HH] + b_hh[0:HH]).astype(np.float32)
    b_z = (b_ih[HH:2 * HH] + b_hh[HH:2 * HH]).astype(np.float32)
    b_in = b_ih[2 * HH:3 * HH].astype(np.float32)
    b_hn = b_hh[2 * HH:3 * HH].astype(np.float32)
    asb = lambda b: np.ascontiguousarray(b.reshape(GJ, P).T)

    Wb = _blocked_w(Wt, KT, GJ)                              # [GJ,P,KT,3,P]
    wh16 = Wb.astype(np.float16)
    wl = Wb - wh16.astype(np.float32)
    wh_scaled = (wh16.astype(np.float32) * SW_H).astype(np.float16)
    # fp8 correction factors only for the z/n gates (g=1,2)
    w8 = np.empty((Wb.shape[0], P, KT, 2, 2, P), dtype=FP8)
    w8[:, :, :, :, 0, :] = (wh16[:, :, :, 1:3].astype(np.float32)
                            * SW8_H).astype(FP8)
    w8[:, :, :, :, 1, :] = (wl[:, :, :, 1:3] * SW8_L).astype(FP8)
    return {"wh": wh_scaled, "w8": w8,
            "br": asb(-b_r * SCALE), "bz": asb(-b_z * SCALE),
            "bin": asb(b_in), "bhn": asb(b_hn)}


def prep_x(x_core, KT, BT):
    """x_core: [BL, I, T] fp32 -> per-core input dict.

    Moving-column layout: col = n*512 + t*BQ + blo with b = n*BQ + blo.
    """
    II = KT * P
    NT = BT // BQ
    a = x_core[:, :II, :].reshape(NT, BQ, II, 4)       # [n, blo, i, t]
    a = a.transpose(2, 0, 3, 1).reshape(II, NT, 512)   # [i, n, t*BQ+blo]
    xt = np.ascontiguousarray(
        a.reshape(KT, P, NT, 512).transpose(2, 1, 0, 3))  # [NT, P, KT, 512]
    xh16 = xt.astype(np.float16)
    xl = xt - xh16.astype(np.float32)
    xh_scaled = (xh16.astype(np.float32) * SX_H).astype(np.float16)
    x8 = (xl * SX8_L).astype(FP8)          # lo piece; hi derived on-chip
    return {"xh": xh_scaled, "x8": x8}


def unpack_out(out, GJ, BT):
    """out: [GJ, NT, P, 3*BQ] fp32 -> spikes [BL, H', 4] with t=0 zeros."""
    HH = GJ * P
    NT = BT // BQ
    arr = out.reshape(GJ, NT, P, 3, BQ)              # [j, n, p, t-1, blo]
    res = np.zeros((BT, HH, 4), dtype=np.float32)
    # res[n*BQ+blo, j*P+p, 1+ti] = arr[j, n, p, ti, blo]
    res[:, :, 1:4] = arr.transpose(1, 4, 0, 2, 3).reshape(BT, HH, 3)
    return res


def kernel(inputs, W_ih, b_ih, W_hh, b_hh):
    from concourse.bass_utils import run_bass_kernel_spmd

    # BT = batch rows per timestep per core (= local batch size BL)
    KT, GJ, BT = I // P, H // P, B // NCORES
    key = (KT, GJ, BT, SCHEME)
    if key not in _CACHE:
        _CACHE[key] = build_nc(KT, GJ, BT)
    nc = _CACHE[key]

    wmap = prep_weights(np.asarray(W_ih, dtype=np.float32),
                        np.asarray(b_ih, dtype=np.float32),
                        np.asarray(b_hh, dtype=np.float32), KT, GJ)

    x = np.asarray(inputs, dtype=np.float32)
    in_maps = []
    BL = B // NCORES
    for c in range(NCORES):
        m = dict(wmap)
        m.update(prep_x(x[c * BL:(c + 1) * BL], KT, BT))
        in_maps.append(m)

    res = run_bass_kernel_spmd(nc, in_maps, list(range(NCORES)), trace=TRACE)
    global LAST_EXEC_NS, LAST_RESULTS
    LAST_EXEC_NS = res.exec_time_ns
    LAST_RESULTS = res

    out = np.empty((B, H, T), dtype=np.float32)
    for c in range(NCORES):
        out[c * BL:(c + 1) * BL] = unpack_out(res.results[c]["out"], GJ, BT)
    return out
